# revision 45
# baseline (speedup 1.0000x reference)
"""GCN pipeline (proj + 2x GCNConv + GraphNorm + spot-softmax aggregation +
MLP head) on 8 trn2 NeuronCores via Bass/Tile.

Sharding: core c owns nodes [c*NSH,(c+1)*NSH) and spots [c*SSH,(c+1)*SSH).
Activations are feature-major [HID, NSH] in SBUF. Each GCN layer builds a
node-major bf16 gather table t' = dinv*(h@W) (256B rows), AllGathers it,
then dst-sorted edges are gathered by src (dma_gather, 4 SWDGE queues) and
scatter-accumulated per dst tile with one-hot matmuls on the PE (gathered
rows stationary -> feature-major PSUM, one open/close per dst tile across
both src halves). Spot attention aggregates [h*e | e] rows (e = exp(score))
with the same gather+one-hot machinery over membership lists sorted by spot.
"""
import sys, os
sys.path.insert(0, '/opt/trn_rl_repo')
import numpy as np

N_CORES = 8
HALF_BITS = 15  # int16 gather idx limit => split tables in two halves


class Cfg:
    def __init__(self, n_nodes=50000, n_edges=800000, in_dim=128, hid=96,
                 attn_hid=32, out_dim=16, n_spots=5000, eps=1e-5):
        assert n_nodes % N_CORES == 0 and n_spots % N_CORES == 0
        self.N, self.E, self.IN, self.H = n_nodes, n_edges, in_dim, hid
        self.AH, self.OD, self.S, self.EPS = attn_hid, out_dim, n_spots, eps
        self.NSH = n_nodes // N_CORES
        self.SSH = n_spots // N_CORES
        self.NT = (self.NSH + 127) // 128          # node tiles per core
        self.ST = (self.SSH + 127) // 128          # spot groups per core
        self.HALF = n_nodes // 2                   # src half split point
        assert self.HALF < (1 << HALF_BITS)
        self.TROWB = 128                           # table row bf16 (256B)
        self.CALL = 2048                           # gather slots per call
        self.BPC = self.CALL // 128                # blocks per call


def _wrap_idx(flat):
    """int16 slot list -> [128, n/16] wrapped layout (replicated 8x)."""
    n = len(flat)
    assert n % 16 == 0
    w = flat.reshape(n // 16, 16).T.astype(np.int16)   # [16, n/16]
    return np.tile(w, (8, 1))


def _split_src(cfg, src):
    """Map global src ids to (table half, in-half idx) for int16 gathers."""
    h = (src >= cfg.HALF).astype(np.int64)
    return src - h * cfg.HALF, h


def _prep_scatter(cfg, ids, dst_local, n_groups):
    """Shared gather/scatter structure builder, uniform across cores.

    ids: per-core global src/node ids; dst_local: per-core local dst/spot
    ids. Blocks are laid out per phase (src rowgroup): groups in order,
    S[g,h] 128-slot blocks each, padded to the max count over cores.
    Returns wrapped idx arrays, dst-local block columns, and meta.
    """
    cnt_all = np.zeros((N_CORES, n_groups, 2), np.int64)
    per_core = []
    for c in range(N_CORES):
        sub, h_c = _split_src(cfg, ids[c])
        d_c = dst_local[c]
        key = d_c // 128 * 2 + h_c
        order = np.argsort(key, kind='stable')
        per_core.append((sub[order], d_c[order], key[order]))
        cnt_all[c] = np.bincount(key, minlength=n_groups * 2)\
            .reshape(n_groups, 2)
    S = (cnt_all.max(axis=0) + 127) // 128             # [G, 2] blocks
    nblk = [int(S[:, h].sum()) for h in (0, 1)]
    slots = [n * 128 for n in nblk]
    b0 = np.zeros((n_groups, 2), np.int64)
    for h in (0, 1):
        b0[:, h] = np.cumsum(S[:, h]) - S[:, h]
    idx_w, dl_w = [], []
    for c in range(N_CORES):
        s_c, d_c, key = per_core[c]
        bounds = np.searchsorted(key, np.arange(n_groups * 2 + 1))
        idx_flat = np.zeros(slots[0] + slots[1], np.int64)
        dl_flat = -np.ones(slots[0] + slots[1], np.float32)
        for g in range(n_groups):
            for h in (0, 1):
                lo, hi = bounds[g * 2 + h], bounds[g * 2 + h + 1]
                n = hi - lo
                off = (0 if h == 0 else slots[0]) + int(b0[g, h]) * 128
                idx_flat[off:off + n] = s_c[lo:hi]
                dl_flat[off:off + n] = (d_c[lo:hi] % 128).astype(np.float32)
        idx_w.append(_wrap_idx(idx_flat))
        dl_w.append(np.ascontiguousarray(
            dl_flat.reshape(-1, 128).T))           # [128, nblk_tot]
    return idx_w, dl_w, dict(S=S, b0=b0, nblk=nblk, slots=slots)


def _prep_edges(cfg, src, dst):
    NSH = cfg.NSH
    core_of = dst // NSH
    ids, dls = [], []
    for c in range(N_CORES):
        m = core_of == c
        ids.append(src[m])
        dls.append(dst[m] - c * NSH)
    return _prep_scatter(cfg, ids, dls, cfg.NT)


def _prep_spots(cfg, cts):
    SSH = cfg.SSH
    nodes = np.arange(cfg.N)
    core_of = cts // SSH
    ids, dls = [], []
    for c in range(N_CORES):
        m = core_of == c
        ids.append(nodes[m])
        dls.append(cts[m] - c * SSH)
    return _prep_scatter(cfg, ids, dls, cfg.ST)


def _calls(total_slots, call):
    out = []
    o = 0
    while o < total_slots:
        n = min(call, total_slots - o)
        out.append((o, n))
        o += n
    return out


def build_program(cfg, emeta, smeta):
    from concourse import bacc, mybir, tile

    f32, i16 = mybir.dt.float32, mybir.dt.int16
    bf16 = mybir.dt.bfloat16
    H, AH, OD = cfg.H, cfg.AH, cfg.OD
    NSH, NT, SSH, ST = cfg.NSH, cfg.NT, cfg.SSH, cfg.ST
    TROWB, BPC = cfg.TROWB, cfg.BPC
    S, b0, nblk, slots = emeta['S'], emeta['b0'], emeta['nblk'], emeta['slots']
    Ssp, b0sp = smeta['S'], smeta['b0']
    nblksp, slotssp = smeta['nblk'], smeta['slots']

    nc = bacc.Bacc("TRN2", target_bir_lowering=False, debug=False,
                   num_devices=N_CORES, num_swdge_queues=4)

    def din(name, shape, dt=f32):
        return nc.dram_tensor(name, shape, dt, kind="ExternalInput")

    xT = din("xT", [cfg.IN, NSH])
    dinv_pp_in = din("dinv_pp", [128, NT])
    dinv_bc_in = din("dinv_bc", [H, NSH])
    iota16_in = din("iota16_in", [128, 128], bf16)
    ident_in = din("ident_in", [128, 128])
    idx_gcn = din("idx_gcn", [128, (slots[0] + slots[1]) // 16], i16)
    dl_gcn = din("dl_gcn", [128, nblk[0] + nblk[1]], bf16)
    idx_spot = din("idx_spot", [128, (slotssp[0] + slotssp[1]) // 16], i16)
    dl_spot = din("dl_spot", [128, nblksp[0] + nblksp[1]], bf16)
    projW = din("projW", [cfg.IN, H])
    W1, W2 = din("W1", [H, H]), din("W2", [H, H])
    attnW1, attnW2 = din("attnW1", [H, AH]), din("attnW2", [AH, 1])
    mlpW1, mlpW2 = din("mlpW1", [H, H]), din("mlpW2", [H, OD])
    # per-feature params packed [96, n]: cols = proj_b, gn0(w,b,a),
    # gcn1_b, gn1(w,b,a), gcn2_b, gn2(w,b,a), mlp_b1, mlpgn(w,b,a)
    pf = din("pf", [H, 16])
    attn_b1 = din("attn_b1", [AH, 1])
    attn_b2 = din("attn_b2", [1, 1])
    mlp_b2 = din("mlp_b2", [OD, 1])
    out = nc.dram_tensor("out", [SSH, OD], f32, kind="ExternalOutput")
    DEBUG = os.environ.get('KERNEL_DEBUG', '0') == '1'
    if DEBUG:
        dbg_h0 = nc.dram_tensor("dbg_h0", [H, NSH], f32, kind="ExternalOutput")
        dbg_h1 = nc.dram_tensor("dbg_h1", [H, NSH], f32, kind="ExternalOutput")
        dbg_h2 = nc.dram_tensor("dbg_h2", [H, NSH], f32, kind="ExternalOutput")
        dbg_sc = nc.dram_tensor("dbg_sc", [1, NSH], f32, kind="ExternalOutput")
        dbg_sp = nc.dram_tensor("dbg_sp", [H, ST * 128], f32,
                                kind="ExternalOutput")
        dbg_ag = [nc.dram_tensor(f"dbg_ag{l}", [H, NSH], f32,
                                 kind="ExternalOutput") for l in range(2)]

    gcalls = [_calls(slots[0], cfg.CALL), _calls(slots[1], cfg.CALL)]
    scalls = [_calls(slotssp[0], cfg.CALL), _calls(slotssp[1], cfg.CALL)]

    with tile.TileContext(nc) as tc:
        with (
            tc.tile_pool(name="res", bufs=1) as res,       # persistent
            tc.tile_pool(name="gatA", bufs=3) as gatA,
            tc.tile_pool(name="gatB", bufs=3) as gatB,
            tc.tile_pool(name="ohpA", bufs=3) as ohpA,
            tc.tile_pool(name="ohpB", bufs=3) as ohpB,
            tc.tile_pool(name="stg", bufs=4) as stg,       # small staging
            tc.tile_pool(name="spp", bufs=2) as spp,       # spot tiles
            tc.tile_pool(name="mmp", bufs=2, space="PSUM") as mmp,
            tc.tile_pool(name="scp", bufs=3, space="PSUM") as scp,
            tc.tile_pool(name="dram", bufs=1, space="DRAM") as dram,
        ):
            # ---------- persistent SBUF ----------
            h = res.tile([128, NSH], f32, name="h_act")       # rows 0:H+1
            agg = res.tile([H, NSH], f32, name="agg")
            dinv_bc = res.tile([H, NSH], f32, name="dinv_bc")
            dinv_pp = res.tile([128, NT], f32, name="dinv_pp")
            iota16 = res.tile([128, 128], bf16, name="iota16")
            ident = res.tile([128, 128], f32, name="ident")
            idxg = res.tile([128, (slots[0] + slots[1]) // 16], i16,
                            name="idxg")
            dlg = res.tile([128, nblk[0] + nblk[1]], bf16, name="dlg")
            idxs_sp = res.tile([128, (slotssp[0] + slotssp[1]) // 16], i16,
                               name="idxs_sp")
            dls_sp = res.tile([128, nblksp[0] + nblksp[1]], bf16,
                              name="dls_sp")
            wproj = res.tile([cfg.IN, H], f32, name="wproj")
            w1 = res.tile([H, H], f32, name="w1")
            w2 = res.tile([H, H], f32, name="w2")
            wa1 = res.tile([H, AH], f32, name="wa1")
            wa2 = res.tile([AH, 1], f32, name="wa2")
            wm1 = res.tile([H, H], f32, name="wm1")
            wm2 = res.tile([H, OD], f32, name="wm2")
            pft = res.tile([H, 16], f32, name="pft")
            ab1 = res.tile([AH, 1], f32, name="ab1")
            ab2 = res.tile([1, 1], f32, name="ab2")
            mb2 = res.tile([OD, 1], f32, name="mb2")
            sq = res.tile([H, 512], f32, name="sq")           # square scratch
            vec = res.tile([H, 8], f32, name="vec")           # tiny math

            for i_, (t_, s_) in enumerate((
                    (iota16, iota16_in), (ident, ident_in),
                    (idxg, idx_gcn), (dlg, dl_gcn),
                    (idxs_sp, idx_spot), (dls_sp, dl_spot),
                    (wproj, projW), (w1, W1), (w2, W2),
                    (wa1, attnW1), (wa2, attnW2), (wm1, mlpW1),
                    (wm2, mlpW2), (pft, pf), (ab1, attn_b1),
                    (ab2, attn_b2), (mb2, mlp_b2),
                    (dinv_pp, dinv_pp_in), (dinv_bc, dinv_bc_in))):
                (nc.sync if i_ % 2 == 0 else nc.scalar).dma_start(t_[:], s_[:])

            # DRAM: tables + collective bounces
            tbl_own = [dram.tile([NSH, TROWB], bf16, name=f"tblo{i}")
                       for i in range(3)]
            tbl_full = [dram.tile([cfg.N, TROWB], bf16, addr_space="Shared",
                                  name=f"tblf{i}") for i in range(3)]
            st_in = [dram.tile([H, 2], f32, name=f"sti{i}") for i in range(4)]
            st_out = [dram.tile([H, 2], f32, addr_space="Shared",
                                name=f"sto{i}") for i in range(4)]

            NCHUNK = (NSH + 511) // 512

            def tsz(t):
                return min(128, NSH - t * 128)

            def csz(ci):
                return min(512, NSH - ci * 512)

            def graph_norm_relu(dst_ap, u_ap, width, n_total, stats_idx,
                                pre_b_col, gn_cols, stats_pre=None,
                                apply_chunked=False):
                """dst = relu(S*u + B) with GN stats over u[:, :width].

                u is the pre-GN input WITHOUT the preceding linear bias
                (pre_b_col, a pf column or None); stats/affine fold it in.
                stats_pre: optional ([H, k] sum-partials, [H, k] sq-partials)
                computed upstream (per-tile, pipelined with the scatter).
                """
                s1 = vec[:, 0:1]
                if stats_pre is not None:
                    s1p_, s2p_ = stats_pre
                    nc.vector.tensor_reduce(s1, s1p_,
                                            mybir.AxisListType.X,
                                            mybir.AluOpType.add)
                    nc.vector.tensor_reduce(vec[:, 1:2], s2p_,
                                            mybir.AxisListType.X,
                                            mybir.AluOpType.add)
                else:
                    nc.vector.tensor_reduce(s1, u_ap[:, :width],
                                            mybir.AxisListType.X,
                                            mybir.AluOpType.add)
                    nch = (width + 511) // 512
                    s2p = res.tile([H, nch], f32, name=f"s2p{stats_idx}")
                    for ci in range(nch):
                        w_ = min(512, width - ci * 512)
                        nc.scalar.activation(
                            sq[:, :w_], u_ap[:, ci * 512:ci * 512 + w_],
                            mybir.ActivationFunctionType.Square,
                            accum_out=s2p[:, ci:ci + 1])
                    nc.vector.tensor_reduce(vec[:, 1:2], s2p[:],
                                            mybir.AxisListType.X,
                                            mybir.AluOpType.add)
                stv = stg.tile([H, 2], f32, name=f"stv{stats_idx}")
                nc.vector.tensor_copy(stv[:], vec[:, 0:2])
                nc.sync.dma_start(st_in[stats_idx][:], stv[:])
                with tc.high_priority():
                    nc.gpsimd.collective_compute(
                        "AllReduce", mybir.AluOpType.add,
                        replica_groups=[list(range(N_CORES))],
                        ins=[st_in[stats_idx][:].opt()],
                        outs=[st_out[stats_idx][:].opt()])
                stt = stg.tile([H, 2], f32, name=f"stt{stats_idx}")
                nc.sync.dma_start(stt[:], st_out[stats_idx][:])
                gw = pft[:, gn_cols[0]:gn_cols[0] + 1]
                gb = pft[:, gn_cols[1]:gn_cols[1] + 1]
                ga = pft[:, gn_cols[2]:gn_cols[2] + 1]
                mean = vec[:, 2:3]
                ex2 = vec[:, 3:4]
                inv_n = 1.0 / float(n_total)
                nc.vector.tensor_scalar(mean, stt[:, 0:1], inv_n, None,
                                        mybir.AluOpType.mult)
                nc.vector.tensor_scalar(ex2, stt[:, 1:2], inv_n, None,
                                        mybir.AluOpType.mult)
                if pre_b_col is not None:
                    c_ = pft[:, pre_b_col:pre_b_col + 1]
                    # mean_x = mean + c ; ex2_x = ex2 + 2*c*mean + c^2
                    t0 = vec[:, 4:5]
                    nc.vector.tensor_tensor(t0, c_, mean, mybir.AluOpType.mult)
                    nc.vector.tensor_scalar(t0, t0, 2.0, None,
                                            mybir.AluOpType.mult)
                    nc.vector.tensor_tensor(ex2, ex2, t0, mybir.AluOpType.add)
                    t1 = vec[:, 5:6]
                    nc.vector.tensor_tensor(t1, c_, c_, mybir.AluOpType.mult)
                    nc.vector.tensor_tensor(ex2, ex2, t1, mybir.AluOpType.add)
                    nc.vector.tensor_tensor(mean, mean, c_, mybir.AluOpType.add)
                # var = ex2 - mean^2 * a * (2 - a)
                m2 = vec[:, 4:5]
                nc.vector.tensor_tensor(m2, mean, mean, mybir.AluOpType.mult)
                a2 = vec[:, 5:6]
                nc.vector.tensor_scalar(a2, ga, -1.0, 2.0,
                                        mybir.AluOpType.mult,
                                        mybir.AluOpType.add)  # 2 - a
                nc.vector.tensor_tensor(a2, a2, ga, mybir.AluOpType.mult)
                nc.vector.tensor_tensor(m2, m2, a2, mybir.AluOpType.mult)
                var = vec[:, 6:7]
                nc.vector.tensor_tensor(var, ex2, m2,
                                        mybir.AluOpType.subtract)
                nc.vector.tensor_scalar(var, var, float(cfg.EPS), None,
                                        mybir.AluOpType.add)
                nc.scalar.activation(var, var,
                                     mybir.ActivationFunctionType.Sqrt)
                nc.vector.reciprocal(var, var)               # rs
                Sg = vec[:, 4:5]
                nc.vector.tensor_tensor(Sg, gw, var, mybir.AluOpType.mult)
                Bg = vec[:, 5:6]
                nc.vector.tensor_tensor(Bg, Sg, ga, mybir.AluOpType.mult)
                nc.vector.tensor_tensor(Bg, Bg, mean, mybir.AluOpType.mult)
                nc.vector.tensor_tensor(Bg, gb, Bg, mybir.AluOpType.subtract)
                if pre_b_col is not None:
                    c_ = pft[:, pre_b_col:pre_b_col + 1]
                    t0 = vec[:, 6:7]
                    nc.vector.tensor_tensor(t0, Sg, c_, mybir.AluOpType.mult)
                    nc.vector.tensor_tensor(Bg, Bg, t0, mybir.AluOpType.add)
                if apply_chunked:
                    for ci in range((width + 511) // 512):
                        w_ = min(512, width - ci * 512)
                        nc.scalar.activation(
                            dst_ap[:, ci * 512:ci * 512 + w_],
                            u_ap[:, ci * 512:ci * 512 + w_],
                            mybir.ActivationFunctionType.Relu,
                            bias=Bg, scale=Sg)
                else:
                    nc.scalar.activation(dst_ap, u_ap,
                                         mybir.ActivationFunctionType.Relu,
                                         bias=Bg, scale=Sg)

            def gather_phase(tbl, idx_tile, dl_tile, calls, meta,
                             n_groups, close_fn, tag, g_stationary):
                """Issue gathers + one-hots, consume per group with a single
                PSUM open across both src halves.

                g_stationary: True -> matmul(ps, g_rows, onehot) giving
                feature-major ps [H, 128]; False -> matmul(ps, onehot,
                g_rows) giving group-major ps [128, H+1] (spot path).
                """
                S_, b0_ = meta['S'], meta['b0']
                slots_, nblk_ = meta['slots'], meta['nblk']
                qn = 0
                tiles_g = [[None] * len(calls[0]), [None] * len(calls[1])]
                tiles_oh = [[None] * len(calls[0]), [None] * len(calls[1])]
                for k in range(max(len(calls[0]), len(calls[1]))):
                    for hph in (0, 1):
                        if k >= len(calls[hph]):
                            continue
                        o, n = calls[hph][k]
                        nb = n // 128
                        col0 = 0 if hph == 0 else slots_[0] // 16
                        blk0 = 0 if hph == 0 else nblk_[0]
                        tview = tbl[hph * cfg.HALF:
                                    hph * cfg.HALF + cfg.HALF, :]
                        pool = gatA if hph == 0 else gatB
                        g = pool.tile([128, BPC, TROWB], bf16,
                                      name=f"g{tag}_{hph}_{k}",
                                      tag=f"gat{hph}")
                        nc.gpsimd.dma_gather(
                            g[:, :nb, :], tview,
                            idx_tile[:, col0 + o // 16: col0 + (o + n) // 16],
                            n, n, TROWB, single_packet=False, queue_num=qn)
                        qn = (qn + 1) % 4
                        opool = ohpA if hph == 0 else ohpB
                        oh = opool.tile([128, BPC, 128], bf16,
                                        name=f"oh{tag}_{hph}_{k}",
                                        tag=f"oh{hph}")
                        dlsl = dl_tile[:, blk0 + o // 128:
                                       blk0 + (o + n) // 128]
                        nc.vector.tensor_tensor(
                            oh[:, :nb, :],
                            iota16[:].unsqueeze(1).broadcast_to([128, nb, 128]),
                            dlsl.unsqueeze(2).broadcast_to([128, nb, 128]),
                            mybir.AluOpType.is_equal)
                        tiles_g[hph][k] = g
                        tiles_oh[hph][k] = oh
                for gi in range(n_groups):
                    nb_tot = int(S_[gi, 0] + S_[gi, 1])
                    ps = scp.tile([128, 128], f32, name=f"ps{tag}_{gi}",
                                  tag="sc")
                    done = 0
                    for hph in (0, 1):
                        for b in range(int(b0_[gi, hph]),
                                       int(b0_[gi, hph] + S_[gi, hph])):
                            k, j = b // BPC, b % BPC
                            g = tiles_g[hph][k]
                            oh = tiles_oh[hph][k]
                            first = done == 0
                            last = done == nb_tot - 1
                            if g_stationary:
                                nc.tensor.matmul(ps[:H, :], g[:, j, :H],
                                                 oh[:, j, :],
                                                 start=first, stop=last)
                            else:
                                nc.tensor.matmul(ps[:, :H + 1], oh[:, j, :],
                                                 g[:, j, :H + 1],
                                                 start=first, stop=last)
                            done += 1
                    close_fn(gi, ps, True, True)

            # ================= proj layer =================
            nc.sync.dma_start(h[:cfg.IN, :], xT[:])
            for ci in range(NCHUNK):
                w_ = csz(ci)
                ps = mmp.tile([H, 512], f32, name=f"pj{ci}", tag="mm")
                nc.tensor.matmul(ps[:, :w_], wproj[:],
                                 h[:cfg.IN, ci * 512:ci * 512 + w_],
                                 start=True, stop=True)
                nc.vector.tensor_copy(agg[:, ci * 512:ci * 512 + w_],
                                      ps[:, :w_])
            # pf cols: 0=proj_b, (1,2,3)=gn0, 4=gcn1_b, (5,6,7)=gn1,
            #          8=gcn2_b, (9,10,11)=gn2, 12=mlp_b1, (13,14,15)=mlpgn
            graph_norm_relu(h[:H, :], agg[:], NSH, cfg.N, 0, 0, (1, 2, 3))
            if DEBUG:
                nc.sync.dma_start(dbg_h0[:], h[:H, :])

            # ================= GCN layers =================
            def build_table(li, write_fn):
                """Per-tile table rows -> DRAM table + AllGather.

                Table-write DMAs alternate between the two HWDGE sequencers
                (sync/scalar) so descriptor generation is not serialized on
                one sequencer.
                """
                for t in range(NT):
                    n_ = tsz(t)
                    sg, cols = write_fn(t, n_)
                    eng = nc.sync if t % 2 == 0 else nc.scalar
                    eng.dma_start(
                        tbl_own[li][t * 128:t * 128 + n_, :cols],
                        sg[:n_, :cols])
                with tc.high_priority():
                    nc.gpsimd.collective_compute(
                        "AllGather", mybir.AluOpType.bypass,
                        replica_groups=[list(range(N_CORES))],
                        ins=[tbl_own[li][:].opt()],
                        outs=[tbl_full[li][:].opt()])

            for li, (Wt, b_col, gn_cols) in enumerate(
                    ((w1, 4, (5, 6, 7)), (w2, 8, (9, 10, 11)))):
                # table t' = dinv * (h @ W), node-major bf16 256B rows
                def tbl_write(t, n_, _Wt=Wt, _li=li):
                    ps = mmp.tile([128, H], f32, name=f"tb{_li}_{t}",
                                  tag="mm")
                    nc.tensor.matmul(ps[:n_, :], h[:H, t * 128:t * 128 + n_],
                                     _Wt[:], start=True, stop=True)
                    sg = stg.tile([128, H], bf16, name=f"ts{_li}_{t}",
                                  tag="tstg")
                    nc.vector.tensor_scalar(sg[:n_, :], ps[:n_, :],
                                            dinv_pp[:n_, t:t + 1], None,
                                            mybir.AluOpType.mult)
                    return sg, H

                build_table(li, tbl_write)
                # self-loop term: agg = dinv_bc * (h @ W)
                for ci in range(NCHUNK):
                    w_ = csz(ci)
                    ps = mmp.tile([H, 512], f32, name=f"sf{li}_{ci}", tag="mm")
                    nc.tensor.matmul(ps[:, :w_], Wt[:],
                                     h[:H, ci * 512:ci * 512 + w_],
                                     start=True, stop=True)
                    nc.vector.tensor_tensor(
                        agg[:, ci * 512:ci * 512 + w_], ps[:, :w_],
                        dinv_bc[:, ci * 512:ci * 512 + w_],
                        mybir.AluOpType.mult)

                s1p = res.tile([H, NT], f32, name=f"s1p_l{li}")
                s2p = res.tile([H, NT], f32, name=f"s2p_l{li}")

                def gcn_close(t, ps, is_first, is_last, _s1p=s1p, _s2p=s2p):
                    # accumulate scatter; on the last phase also apply the
                    # dinv scale + per-tile stats so the stats AllReduce can
                    # fire right after the last tile closes.
                    n_ = tsz(t)
                    a_ = agg[:, t * 128:t * 128 + n_]
                    nc.vector.tensor_tensor(a_, a_, ps[:H, :n_],
                                            mybir.AluOpType.add)
                    if not is_last:
                        return
                    nc.vector.tensor_tensor(
                        a_, a_, dinv_bc[:, t * 128:t * 128 + n_],
                        mybir.AluOpType.mult)
                    nc.vector.tensor_reduce(_s1p[:, t:t + 1], a_,
                                            mybir.AxisListType.X,
                                            mybir.AluOpType.add)
                    nc.scalar.activation(sq[:, :n_], a_,
                                         mybir.ActivationFunctionType.Square,
                                         accum_out=_s2p[:, t:t + 1])

                gather_phase(tbl_full[li], idxg, dlg, gcalls,
                             emeta, NT, gcn_close, f"e{li}", True)
                if DEBUG:
                    nc.sync.dma_start(dbg_ag[li][:], agg[:])
                graph_norm_relu(h[:H, :], agg[:], NSH, cfg.N,
                                1 + li, b_col, gn_cols,
                                stats_pre=(s1p[:], s2p[:]),
                                apply_chunked=True)
                if DEBUG:
                    nc.sync.dma_start((dbg_h1 if li == 0 else dbg_h2)[:],
                                      h[:H, :])

            # ================= attention scores =================
            # u_att = relu(attn_W1.T @ h + b1); score = attn_W2.T @ u + b2
            for ci in range(NCHUNK):
                w_ = csz(ci)
                ps = mmp.tile([AH, 512], f32, name=f"at{ci}", tag="mm")
                nc.tensor.matmul(ps[:, :w_], wa1[:],
                                 h[:H, ci * 512:ci * 512 + w_],
                                 start=True, stop=True)
                uc = stg.tile([AH, 512], f32, name=f"uat{ci}", tag="uat")
                nc.scalar.activation(uc[:, :w_], ps[:, :w_],
                                     mybir.ActivationFunctionType.Relu,
                                     bias=ab1[:])
                ps2 = mmp.tile([1, 512], f32, name=f"sc2{ci}", tag="tpose")
                nc.tensor.matmul(ps2[:, :w_], wa2[:], uc[:, :w_],
                                 start=True, stop=True)
                nc.vector.tensor_scalar(h[H:H + 1, ci * 512:ci * 512 + w_],
                                        ps2[:, :w_], ab2[:],
                                        None, mybir.AluOpType.add)

            if DEBUG:
                nc.sync.dma_start(dbg_sc[:], h[H:H + 1, :])

            # spot table rows: [h*e (H) | e | pad], e = exp(score)
            def spot_write(t, n_):
                ps = mmp.tile([128, 128], f32, name=f"tr{t}", tag="tpose")
                nc.tensor.transpose(ps[:n_, :H + 1],
                                    h[:H + 1, t * 128:t * 128 + n_],
                                    ident[:H + 1, :H + 1])
                ec = stg.tile([128, 1], f32, name=f"ec{t}", tag="ec")
                nc.scalar.activation(ec[:n_], ps[:n_, H:H + 1],
                                     mybir.ActivationFunctionType.Exp)
                sg = stg.tile([128, H + 1], bf16, name=f"ts2_{t}", tag="tstg")
                nc.vector.tensor_scalar(sg[:n_, :H], ps[:n_, :H], ec[:n_],
                                        None, mybir.AluOpType.mult)
                nc.vector.tensor_copy(sg[:n_, H:H + 1], ec[:n_])
                return sg, H + 1

            build_table(2, spot_write)

            # ================= spot aggregation =================
            spot_fm = res.tile([H, ST * 128], f32, name="spot_fm")
            sacc = [res.tile([128, H + 1], f32, name=f"sacc{gi}")
                    for gi in range(ST)]

            def spot_close(gi, ps, is_first, is_last):
                if is_first and not is_last:
                    nc.vector.tensor_copy(sacc[gi][:], ps[:, :H + 1])
                    return
                if not is_first:
                    nc.vector.tensor_tensor(sacc[gi][:], sacc[gi][:],
                                            ps[:, :H + 1],
                                            mybir.AluOpType.add)
                src = ps[:, :H + 1] if (is_first and is_last) else sacc[gi][:]
                den = spp.tile([128, 1], f32, name=f"den{gi}", tag="den")
                nc.vector.tensor_scalar(den[:], src[:, H:H + 1], 1e-30, None,
                                        mybir.AluOpType.max)
                nc.vector.reciprocal(den[:], den[:])
                sv = spp.tile([128, H], f32, name=f"sv{gi}", tag="sv")
                nc.vector.tensor_scalar(sv[:], src[:, :H], den[:], None,
                                        mybir.AluOpType.mult)
                pt = mmp.tile([H, 128], f32, name=f"spt{gi}", tag="tpose")
                nc.tensor.transpose(pt[:], sv[:], ident[:])
                nc.vector.tensor_copy(spot_fm[:, gi * 128:(gi + 1) * 128],
                                      pt[:])

            gather_phase(tbl_full[2], idxs_sp, dls_sp, scalls,
                         smeta, ST, spot_close, "s", False)

            if DEBUG:
                nc.sync.dma_start(dbg_sp[:], spot_fm[:])
            # ================= MLP head =================
            um = res.tile([H, ST * 128], f32, name="um")
            for ci in range((ST * 128 + 511) // 512):
                w_ = min(512, ST * 128 - ci * 512)
                ps = mmp.tile([H, 512], f32, name=f"m1{ci}", tag="mm")
                nc.tensor.matmul(ps[:, :w_], wm1[:],
                                 spot_fm[:, ci * 512:ci * 512 + w_],
                                 start=True, stop=True)
                nc.vector.tensor_copy(um[:, ci * 512:ci * 512 + w_],
                                      ps[:, :w_])
            graph_norm_relu(um[:], um[:], SSH, cfg.S, 3, 12, (13, 14, 15))
            zo = res.tile([OD, ST * 128], f32, name="zo")
            for ci in range((ST * 128 + 511) // 512):
                w_ = min(512, ST * 128 - ci * 512)
                ps = mmp.tile([OD, 512], f32, name=f"m2{ci}", tag="mm")
                nc.tensor.matmul(ps[:, :w_], wm2[:],
                                 um[:, ci * 512:ci * 512 + w_],
                                 start=True, stop=True)
                nc.vector.tensor_scalar(zo[:, ci * 512:ci * 512 + w_],
                                        ps[:, :w_], mb2[:], None,
                                        mybir.AluOpType.add)
            for gi in range(ST):
                n_ = min(128, SSH - gi * 128)
                if n_ <= 0:
                    break
                ps = mmp.tile([128, OD], f32, name=f"ot{gi}", tag="tpose")
                nc.tensor.transpose(ps[:, :], zo[:, gi * 128:(gi + 1) * 128],
                                    ident[:OD, :OD])
                sg = stg.tile([128, OD], f32, name=f"os{gi}", tag="ostg")
                nc.vector.tensor_copy(sg[:], ps[:])
                (nc.sync if gi % 2 == 0 else nc.scalar).dma_start(
                    out[gi * 128:gi * 128 + n_, :], sg[:n_, :])

    nc.compile()
    return nc


_CACHE = {}


def _build_inputs(cfg, inputs, idx_w, dl_w, idxs_w, dls_w, dinv):
    f = np.float32
    import ml_dtypes
    x = np.asarray(inputs['x'], f)
    xT = np.ascontiguousarray(x.T)

    def col(v):
        return np.asarray(v, f).reshape(-1, 1)

    pf = np.zeros((cfg.H, 16), f)
    for i, k in enumerate(['proj_b', 'gn0_w', 'gn0_b', 'gn0_a',
                           'gcn1_b', 'gn1_w', 'gn1_b', 'gn1_a',
                           'gcn2_b', 'gn2_w', 'gn2_b', 'gn2_a',
                           'mlp_b1', 'mlp_gn_w', 'mlp_gn_b', 'mlp_gn_a']):
        pf[:, i] = np.asarray(inputs[k], f)
    iota = np.broadcast_to(np.arange(128, dtype=f), (128, 128)).copy()
    iota16 = iota.astype(ml_dtypes.bfloat16)
    ident = np.eye(128, dtype=f)
    in_maps = []
    for c in range(N_CORES):
        n0 = c * cfg.NSH
        dinv_own = dinv[n0:n0 + cfg.NSH]
        dpp = np.ones((128, cfg.NT), f)
        for t in range(cfg.NT):
            n_ = min(128, cfg.NSH - t * 128)
            dpp[:n_, t] = dinv_own[t * 128:t * 128 + n_]
        dbc = np.broadcast_to(dinv_own[None, :], (cfg.H, cfg.NSH)).copy()
        in_maps.append({
            'xT': np.ascontiguousarray(xT[:, n0:n0 + cfg.NSH]),
            'dinv_pp': dpp, 'dinv_bc': dbc,
            'iota16_in': iota16, 'ident_in': ident,
            'idx_gcn': idx_w[c],
            'dl_gcn': dl_w[c].astype(ml_dtypes.bfloat16),
            'idx_spot': idxs_w[c],
            'dl_spot': dls_w[c].astype(ml_dtypes.bfloat16),
            'projW': np.asarray(inputs['proj_W'], f),
            'W1': np.asarray(inputs['gcn1_W'], f),
            'W2': np.asarray(inputs['gcn2_W'], f),
            'attnW1': np.asarray(inputs['attn_W1'], f),
            'attnW2': np.asarray(inputs['attn_W2'], f),
            'mlpW1': np.asarray(inputs['mlp_W1'], f),
            'mlpW2': np.asarray(inputs['mlp_W2'], f),
            'pf': pf,
            'attn_b1': col(inputs['attn_b1']),
            'attn_b2': col(inputs['attn_b2']),
            'mlp_b2': col(inputs['mlp_b2']),
        })
    return in_maps


def _ensure_ntff_hook():
    """Install the axon NTFF profile hook if the image's antenv lacks it.

    Only used when KERNEL_TRACE=1; failures degrade to no-trace runs.
    """
    try:
        import antenv.axon_hooks  # noqa: F401
        return
    except ImportError:
        pass
    try:
        import types
        import antenv
        from trn_agent_boot.trn_boot import _ntff_profile_via_ctypes
        hook = _ntff_profile_via_ctypes('/opt/axon/libaxon_pjrt.so')
        mod = types.ModuleType('antenv.axon_hooks')
        state = {'hook': hook}
        mod.set_axon_ntff_profile_hook = lambda h: state.__setitem__('hook', h)
        mod.get_axon_ntff_profile_hook = lambda: state['hook']
        sys.modules['antenv.axon_hooks'] = mod
        antenv.axon_hooks = mod
    except Exception:
        pass


def kernel(**inputs):
    from concourse import bass_utils
    if os.environ.get('KERNEL_TRACE', '0') == '1':
        _ensure_ntff_hook()
    cfg = Cfg(n_nodes=int(np.asarray(inputs['x']).shape[0]),
              n_edges=int(np.asarray(inputs['edge_index']).shape[1]),
              in_dim=int(np.asarray(inputs['x']).shape[1]),
              hid=int(np.asarray(inputs['proj_W']).shape[1]),
              attn_hid=int(np.asarray(inputs['attn_W1']).shape[1]),
              out_dim=int(np.asarray(inputs['mlp_W2']).shape[1]),
              n_spots=int(inputs['num_spots']))
    ei = np.asarray(inputs['edge_index']).astype(np.int64)
    cts = np.asarray(inputs['cell_to_spot']).astype(np.int64)
    src, dst = ei[0], ei[1]
    deg = (np.bincount(dst, minlength=cfg.N) + 1).astype(np.float32)
    dinv = (1.0 / np.sqrt(deg)).astype(np.float32)

    idx_w, dl_w, emeta = _prep_edges(cfg, src, dst)
    idxs_w, dls_w, smeta = _prep_spots(cfg, cts)

    key = (cfg.N, cfg.E, tuple(emeta['nblk']), tuple(smeta['nblk']))
    if key not in _CACHE:
        _CACHE[key] = build_program(cfg, emeta, smeta)
    nc = _CACHE[key]

    in_maps = _build_inputs(cfg, inputs, idx_w, dl_w, idxs_w, dls_w, dinv)
    res = bass_utils.run_bass_kernel_spmd(
        nc, in_maps, core_ids=list(range(N_CORES)),
        trace=os.environ.get('KERNEL_TRACE', '0') == '1',
        tmpdir=os.environ.get('KERNEL_TMPD'))
    if os.environ.get('KERNEL_TRACE', '0') == '1':
        print('HW exec time:', res.exec_time_ns, 'ns')
    out = np.concatenate([res.results[c]['out'] for c in range(N_CORES)],
                         axis=0)
    return out.astype(np.float32)
